# revision 18
# baseline (speedup 1.0000x reference)
import sys
import functools

sys.path.insert(0, "/opt/trn_rl_repo")
import numpy as np
import ml_dtypes

# Problem constants (nn_Causal_GraphConvolution): hardcoded per harness contract.
K = 2
N = 8192
IN_F = 128
OUT_F = 64
NCORES = 8
ROWS = N // NCORES   # 1024 attention rows per core
NCH = N // 128       # 64 column chunks of 128
RCH = ROWS // 128    # 8 row chunks of 128
CPG = 4              # adjacency chunks per DMA group
NGRP = NCH // CPG    # 16 adjacency tiles
# chunks whose attention tile is computed on ScalarE (relu path) instead of
# VectorE; balances DVE vs ACT occupancy. chunk ch is on the ACT path when
# greedy engine balance for the attention-tile op (see _act_schedule)
ACT_NS = 1075.0   # ScalarE relu tile cost
DVE_TS_NS = 327.0  # VectorE tensor_scalar tile cost
DVE_TT_NS = 594.0  # VectorE mask-multiply tile cost (always on DVE)
DVE_MISC_NS = 12000.0  # copies/normalize baseline on DVE
ACT_MISC_NS = 4000.0   # exps baseline on ACT


def _act_schedule():
    """on_act[k][ch]: greedy min-makespan assignment of the t-op."""
    act_t, dve_t = ACT_MISC_NS, DVE_MISC_NS
    on_act = [[False] * NCH for _ in range(K)]
    for ch in range(NCH):
        for k in range(K):
            dve_t += DVE_TT_NS
            if ch != NCH - 1 and act_t + ACT_NS < dve_t + DVE_TS_NS:
                on_act[k][ch] = True
                act_t += ACT_NS
            else:
                dve_t += DVE_TS_NS
    return on_act


@functools.lru_cache(maxsize=2)
def _build(timing=False):
    import concourse.bacc as bacc
    import concourse.tile as tile
    from concourse.tile import add_dep_helper
    from concourse import mybir

    bf16 = mybir.dt.bfloat16
    f32 = mybir.dt.float32
    AO = mybir.AluOpType
    AF = mybir.ActivationFunctionType

    nc = bacc.Bacc(num_devices=NCORES)

    # Per-core inputs (the SPMD in_maps supply different data per core).
    # adjT[g, p, j, r] = adj[core_rows[r], (g*CPG+j)*128 + p]
    adjT = nc.declare_dram_parameter("adjT", [NGRP, 128, CPG, ROWS], bf16, False)
    xT = nc.declare_dram_parameter("xT", [K, IN_F, N], bf16, False)
    xrT = nc.declare_dram_parameter("xrT", [K, IN_F, ROWS], bf16, False)
    waug = nc.declare_dram_parameter("waug", [IN_F, 66], bf16, False)
    out = nc.declare_dram_parameter("out", [ROWS, K, OUT_F], f32, True)

    urow = nc.dram_tensor("urow", [K, 1, ROWS], bf16)
    hp_local = [
        nc.dram_tensor(f"hp_local{k}", [ROWS, OUT_F], bf16) for k in range(K)
    ]
    hp_full = [
        nc.dram_tensor(
            f"hp_full{k}", [N, OUT_F], bf16,
            addr_space="Local" if timing else "Shared",
        )
        for k in range(K)
    ]

    with tile.TileContext(nc) as tc:
        with (
            tc.tile_pool(name="persist", bufs=1) as persist,
            tc.tile_pool(name="adjp", bufs=NGRP) as adjp,
            tc.tile_pool(name="xp", bufs=1) as xp,
            tc.tile_pool(name="tp", bufs=4) as tp,
            tc.tile_pool(name="pp", bufs=5) as pp,
            tc.tile_pool(name="hpio", bufs=3) as hpio,
            tc.tile_pool(name="sm", bufs=8) as sm,
        ):
            # phase-0 inputs first so they are not queued behind the 16MB
            # adjacency stream
            waug_sb = persist.tile([IN_F, 66], bf16, tag="waug")
            nc.gpsimd.dma_start(out=waug_sb, in_=waug[:])
            neg1 = persist.tile([128, 1], f32, tag="neg1")
            nc.vector.memset(neg1, -1.0)
            XQ = N // 4
            xbigs, xrs = [], []
            x_dmas = []
            for k in range(K):
                xb_k = []
                for q in range(4):
                    xbig = xp.tile([128, XQ], bf16, tag="xbig", bufs=2,
                                   name=f"xbig{k}_{q}")
                    x_dmas.append(nc.sync.dma_start(
                        out=xbig, in_=xT[k, :, q * XQ:(q + 1) * XQ]
                    ))
                    xb_k.append(xbig)
                xbigs.append(xb_k)
                xr_sb = xp.tile([128, ROWS], bf16, tag="xr", bufs=2,
                                name=f"xr{k}")
                nc.gpsimd.dma_start(out=xr_sb, in_=xrT[k])
                xrs.append(xr_sb)

            # ---- resident adjacency (mask in phase 1, weights in phase 2) ----
            # Chain the group loads (2 in flight) so they LAND in consumption
            # order; unchained, all 16 progress concurrently and group 0
            # completes nearly as late as group 15, stalling phase 1.
            adj_sb = []  # adj_sb[g][:, j, :] is chunk g*CPG+j as [128, ROWS]
            adj_dmas = []
            for g in range(NGRP):
                at = adjp.tile([128, CPG, ROWS], bf16, tag="adjt", name=f"adj{g}")
                d = nc.sync.dma_start(out=at, in_=adjT[g])
                if g >= 2:
                    add_dep_helper(d.ins, adj_dmas[g - 2].ins,
                                   reason="stream adjacency in order")
                else:
                    add_dep_helper(d.ins, x_dmas[-1].ins,
                                   reason="x lands before adjacency stream")
                adj_dmas.append(d)
                adj_sb.append(at)

            def adj_ch(ch):
                return adj_sb[ch // CPG][:, ch % CPG, :]

            whp = []   # [128, NCH, 65] per k: [Wh chunk | ones]
            v_sb = []  # [128, NCH] f32 per k: exp(Wh2)
            u_bc = []  # [128, ROWS] bf16 per k: exp(Wh1[rows]) bcast
            psA_cm = tc.tile_pool(name="psA", bufs=3, space="PSUM")
            psA = psA_cm.__enter__()
            for k in range(K):
                # ---- phase 0b first: u = exp(Wh1[core rows]) broadcast ----
                xr_sb = xrs[k]
                for half in range(2):
                    psu = psA.tile([1, 512], f32, tag="psu",
                                   name=f"psu{k}_{half}")
                    nc.tensor.matmul(
                        psu,
                        lhsT=waug_sb[:, 64:65],
                        rhs=xr_sb[:, half * 512:(half + 1) * 512],
                        start=True,
                        stop=True,
                    )
                    uh = sm.tile([1, 512], bf16, tag="uh", name=f"uh{k}_{half}")
                    nc.scalar.activation(uh, psu, AF.Exp)
                    nc.gpsimd.dma_start(
                        out=urow[k, :, half * 512:(half + 1) * 512], in_=uh
                    )
                ub = persist.tile([128, ROWS], bf16, tag=f"ub{k}")
                nc.gpsimd.dma_start(out=ub, in_=urow[k].to_broadcast((128, ROWS)))
                u_bc.append(ub)

                # ---- phase 0: Wh_aug = x @ [W | W@a1 | W@a2] ----
                whp_k = persist.tile([128, NCH, 65], bf16, tag=f"whp{k}")
                nc.vector.memset(whp_k[:, :, 64:65], 1.0)
                wh2_k = persist.tile([128, NCH], f32, tag=f"wh2{k}")
                for q in range(4):
                    xbig = xbigs[k][q]
                    for cb in range(0, NCH // 4, 4):
                        ps0 = psA.tile([128, 4, 66], f32, tag="ps0",
                                       name=f"ps0_{k}_{q}_{cb}")
                        for j in range(4):
                            ch = cb + j
                            nc.tensor.matmul(
                                ps0[:, j, :],
                                lhsT=xbig[:, ch * 128:(ch + 1) * 128],
                                rhs=waug_sb,
                                start=True,
                                stop=True,
                            )
                        gch = q * (NCH // 4) + cb
                        if (gch // 4) % 2 == 0:
                            nc.vector.tensor_copy(
                                whp_k[:, gch:gch + 4, 0:64], ps0[:, :, 0:64]
                            )
                        else:
                            nc.scalar.copy(
                                whp_k[:, gch:gch + 4, 0:64], ps0[:, :, 0:64]
                            )
                        nc.vector.tensor_copy(wh2_k[:, gch:gch + 4], ps0[:, :, 65])
                whp.append(whp_k)

                # v = exp(Wh2), split so phase 1 unblocks after first half
                v_k = persist.tile([128, NCH], f32, tag=f"v{k}")
                nc.scalar.activation(
                    v_k[:, 0:NCH // 2], wh2_k[:, 0:NCH // 2], AF.Exp
                )
                nc.scalar.activation(
                    v_k[:, NCH // 2:], wh2_k[:, NCH // 2:], AF.Exp
                )
                v_sb.append(v_k)
            psA_cm.__exit__(None, None, None)

            psB_cm = tc.tile_pool(name="psB", bufs=8, space="PSUM")
            psB = psB_cm.__enter__()
            # ---- phase 1 ----
            # p[m, r] = adj[r, m] * max(u[r] v[m], 1); h'^T chunks = p.T @ [Wh|1]
            # DVE path: t = max(u*v, 1) via tensor_scalar, then p = t * adj.
            # ACT path: t = relu(u*v - 1) on ScalarE, q = t * adj, and the
            #   missing "+ adj" term is a second PE accumulation of adj @ Whp.
            act_sched = _act_schedule()
            for k in range(K):
                ps_h = [
                    psB.tile([128, 65], f32, tag="acc", name=f"psh{k}_{i}")
                    for i in range(RCH)
                ]
                hp_acc = hpio.tile([128, RCH, OUT_F], bf16, tag="hpacc", bufs=2,
                                   name=f"hpacc{k}")
                for ch in range(NCH):
                    on_act = act_sched[k][ch]
                    t = tp.tile([128, ROWS], bf16, tag="t", name=f"t{k}_{ch}")
                    if on_act:
                        nc.scalar.activation(
                            t, u_bc[k], AF.Relu,
                            bias=neg1, scale=v_sb[k][:, ch:ch + 1],
                        )
                    else:
                        nc.vector.tensor_scalar(
                            out=t,
                            in0=u_bc[k],
                            scalar1=v_sb[k][:, ch:ch + 1],
                            scalar2=1.0,
                            op0=AO.mult,
                            op1=AO.max,
                        )
                    p = pp.tile([128, ROWS], bf16, tag="p", name=f"p{k}_{ch}")
                    nc.vector.tensor_mul(p, t, adj_ch(ch))
                    for ns in range(RCH):
                        nc.tensor.matmul(
                            ps_h[ns],
                            lhsT=p[:, ns * 128:(ns + 1) * 128],
                            rhs=whp[k][:, ch, :],
                            start=(ch == 0),
                            stop=(ch == NCH - 1),
                        )
                    if on_act:
                        for ns in range(RCH):
                            nc.tensor.matmul(
                                ps_h[ns],
                                lhsT=adj_ch(ch)[:, ns * 128:(ns + 1) * 128],
                                rhs=whp[k][:, ch, :],
                                start=False,
                                stop=False,
                            )
                for ns in range(RCH):
                    rs = sm.tile([128, 1], f32, tag="rs", name=f"rs{k}_{ns}")
                    nc.vector.reciprocal(rs, ps_h[ns][:, 64:65])
                    nc.vector.tensor_scalar_mul(
                        hp_acc[:, ns, :], ps_h[ns][:, 0:64], rs
                    )
                nc.gpsimd.dma_start(
                    out=hp_local[k][:].rearrange("(ns p) o -> p ns o", p=128),
                    in_=hp_acc,
                )
                if timing:
                    nc.gpsimd.dma_start(
                        out=hp_full[k][0:ROWS, :], in_=hp_local[k][:]
                    )
                else:
                    nc.gpsimd.collective_compute(
                        "AllGather",
                        mybir.AluOpType.bypass,
                        replica_groups=[list(range(NCORES))],
                        ins=[hp_local[k][:]],
                        outs=[hp_full[k][:]],
                    )

            # ---- phase 2: out = relu(adj[rows, :] @ h'_full) ----
            ps_o = [
                psB.tile([128, K, OUT_F], f32, tag="acc", name=f"pso{i}")
                for i in range(RCH)
            ]
            out_acc = hpio.tile([128, RCH, K * OUT_F], f32, tag="outacc", bufs=1)
            for g in range(NGRP):
                hpbig = hpio.tile([128, CPG, K, OUT_F], bf16, tag="hpbig",
                                  name=f"hpbig{g}")
                base = g * CPG * 128
                for k in range(K):
                    nc.sync.dma_start(
                        out=hpbig[:, :, k, :],
                        in_=hp_full[k][base:base + CPG * 128, :].rearrange(
                            "(j p) o -> p j o", p=128
                        ),
                    )
                for j in range(CPG):
                    ch = g * CPG + j
                    for rs_ in range(RCH):
                        nc.tensor.matmul(
                            ps_o[rs_],
                            lhsT=adj_sb[g][:, j, rs_ * 128:(rs_ + 1) * 128],
                            rhs=hpbig[:, j, :, :],
                            start=(ch == 0),
                            stop=(ch == NCH - 1),
                        )
            for rs_ in range(RCH):
                nc.vector.tensor_scalar_max(out_acc[:, rs_, :], ps_o[rs_], 0.0)
            nc.sync.dma_start(
                out=out[:].rearrange("(rs p) k o -> p rs (k o)", p=128),
                in_=out_acc,
            )
            psB_cm.__exit__(None, None, None)

    nc.finalize()
    return nc


def _prep_inputs(x, adj, weight, a):
    bf = ml_dtypes.bfloat16
    w32 = weight.astype(np.float32)
    a32 = a.astype(np.float32)
    waug = np.concatenate(
        [w32, w32 @ a32[:OUT_F], w32 @ a32[OUT_F:]], axis=1
    ).astype(bf)  # [128, 66]
    xT = np.ascontiguousarray(x.astype(np.float32).transpose(0, 2, 1)).astype(bf)
    adj_bf = adj.astype(bf)
    in_maps = []
    for c in range(NCORES):
        rows = slice(c * ROWS, (c + 1) * ROWS)
        # [N, ROWS] -> [NGRP, 128, CPG, ROWS]; chunk ch = g*CPG+j sits at
        # adj^T rows ch*128 ... ch*128+128
        adjT_c = (
            np.ascontiguousarray(adj_bf[rows].T)
            .reshape(NGRP, CPG, 128, ROWS)
            .transpose(0, 2, 1, 3)
        )
        adjT_c = np.ascontiguousarray(adjT_c)
        xrT_c = np.ascontiguousarray(xT[:, :, rows])
        in_maps.append({"adjT": adjT_c, "xT": xT, "xrT": xrT_c, "waug": waug})
    return in_maps


def _run(in_maps, trace=False, **kw):
    from concourse.bass_utils import run_bass_kernel_spmd

    nc = _build()
    return run_bass_kernel_spmd(nc, in_maps, list(range(NCORES)), trace=trace, **kw)


def kernel(**inputs):
    x = np.asarray(inputs["x"])
    adj = np.asarray(inputs["adj"])
    weight = np.asarray(inputs["weight"])
    a = np.asarray(inputs["a"])
    in_maps = _prep_inputs(x, adj, weight, a)
    res = _run(in_maps)
    full = np.concatenate(
        [np.asarray(res.results[c]["out"]) for c in range(NCORES)], axis=0
    )  # [N, K, OUT_F]
    return np.ascontiguousarray(full.transpose(1, 0, 2)).astype(np.float32)
